# revision 8
# baseline (speedup 1.0000x reference)
"""DCRNN+GCN fused Trainium2 kernel (8-core SPMD, graph partitioned by dst node).

Math (reference):
  h   = GCNConv(x)            = A_norm @ x @ Wg + bg     (sym-normalized, self-loops)
  Z   = sigmoid([h, Hs] @ Wz + bz)         (Wz = Wz0 + Wz1, DConv K=1)
  R   = sigmoid([h, Hs] @ Wr + br)
  Ht  = tanh([h, Hs*R] @ Wh + bh)
  Hn  = Z*Hs + (1-Z)*Ht
  probs = softmax(relu(Hn) @ Wl + bl)
Key reassociation: A_norm @ (x @ Wg) == (A_norm @ x) @ Wg, so the device
gathers raw x rows per edge and applies Wg after aggregation.

Device layout: everything feat-major ([128 feat partitions, nodes free]) until
the final logits, which come out node-major for the free-axis softmax.

Aggregation: edges (+self-loops) are sorted by dst and bucketed into 128-node
windows (49 per core).  Window entries are split by source-table half (the
dma_gather int16 index limit), each half padded to whole 128-entry subtiles:
CA subtiles from x[:SPLIT], CB from x[SPLIT:].  Two dma_gathers fill
G [128 entries, C=CA+CB, 128 feat]; a host-built sparse selector
S [128 entries, C*128 nodes] (one nonzero = GCN norm coeff per entry row)
turns the scatter-add into C PE matmuls accumulating aggT = G.T @ S in PSUM.
Entry i of a window maps to partition i%128, subtile i//128 (dma_gather's
native order); padded entries gather row 0 with S value 0.
"""

import numpy as np
import sys

for _p in ("/opt/trn_rl_repo",):
    if _p not in sys.path:
        sys.path.insert(0, _p)

P = 128          # partitions / tile edge
N_CORES = 8
F_IN = 128
F_HID = 128
N_CLS = 16


def _preprocess(x, edge_index, edge_weight, hidden1, n_nodes, n_cores):
    """Host-side graph prep: self-loops, sym-norm coeffs, dst-sorted bucketing,
    per-core wrapped int16 gather index tables and selector (S) matrices."""
    src = np.asarray(edge_index)[0].astype(np.int64)
    dst = np.asarray(edge_index)[1].astype(np.int64)
    ew = np.asarray(edge_weight).astype(np.float32)

    loop = np.arange(n_nodes, dtype=np.int64)
    s_all = np.concatenate([src, loop])
    d_all = np.concatenate([dst, loop])
    w_all = np.concatenate([ew, np.ones(n_nodes, np.float32)])

    deg = np.bincount(d_all, weights=w_all, minlength=n_nodes).astype(np.float32)
    dinv = np.where(deg > 0, 1.0 / np.sqrt(deg), 0.0).astype(np.float32)
    norm = (dinv[s_all] * w_all * dinv[d_all]).astype(np.float32)

    npc = n_nodes // n_cores            # nodes per core
    nw = -(-npc // P)                   # windows per core
    npc_pad = nw * P
    split = (n_nodes + 1) // 2          # gather table halves (int16 idx limit)

    core_id = d_all // npc
    dl = d_all % npc
    w_id = dl // P
    nloc = dl % P
    bucket = core_id * nw + w_id
    is_b = (s_all >= split).astype(np.int64)

    order = np.lexsort((is_b, bucket))
    bkt, ib = bucket[order], is_b[order]
    ss, ns, nl = s_all[order], norm[order], nloc[order]

    grp = bkt * 2 + ib
    gc = np.bincount(grp, minlength=n_cores * nw * 2)
    gstart = np.concatenate([[0], np.cumsum(gc)[:-1]])
    rank = np.arange(len(bkt)) - gstart[grp]

    CA = int(-(-gc[0::2].max() // P))
    CB = int(-(-gc[1::2].max() // P))
    C = CA + CB

    col = (bkt % nw) * C + ib * CA + rank // P      # subtile column within core
    part = rank % P
    cid = bkt // nw
    idxval = (ss - ib * split).astype(np.int16)

    K = nw * C
    idx_all = np.zeros((n_cores, K * P), np.int16)
    sval = np.zeros((n_cores, P, K * P), np.float32)
    idx_all[cid, col * P + part] = idxval
    sval[cid, part, col * P + nl] = ns

    # wrap idx lists into 16 partitions per gather call (A and B blocks
    # separately per window), replicated to 128 partitions
    idx_wrap = np.zeros((n_cores, 16, K * 8), np.int16)
    for c in range(n_cores):
        for w in range(nw):
            base = w * C * P
            a = idx_all[c, base:base + CA * P].reshape(-1, 16).T
            idx_wrap[c, :, w * C * 8:w * C * 8 + CA * 8] = a
            if CB:
                b = idx_all[c, base + CA * P:base + C * P].reshape(-1, 16).T
                idx_wrap[c, :, w * C * 8 + CA * 8:(w + 1) * C * 8] = b
    idx_wrap = np.tile(idx_wrap, (1, 8, 1))

    hid = np.asarray(hidden1).astype(np.float32)
    hT = np.zeros((n_cores, P, npc_pad), np.float32)
    for c in range(n_cores):
        hT[c, :, :npc] = hid[c * npc:(c + 1) * npc].T

    return dict(idx=idx_wrap, sval=sval, hT=hT, CA=CA, CB=CB, nw=nw,
                npc=npc, npc_pad=npc_pad, split=split)


def _pack_weights(Wg, bg, Wz0, Wz1, bz, Wr0, Wr1, br, Wh0, Wh1, bh, Wl, bl):
    """Pack gate weights as lhsT chunk pairs: gWs = [zA|zB|rA|rB|hA|hB]."""
    f = F_IN

    def chunks(W):
        W = np.asarray(W).astype(np.float32)
        return [W[:f, :], W[f:, :]]

    gW0 = np.concatenate(chunks(Wz0) + chunks(Wr0) + chunks(Wh0), axis=1)
    gW1 = np.concatenate(chunks(Wz1) + chunks(Wr1) + chunks(Wh1), axis=1)
    gWs = gW0 + gW1     # DConv K=1 applies x @ (W0 + W1); fold on host
    gB = np.stack([np.asarray(b).astype(np.float32)
                   for b in (bg, bz, br, bh)], axis=1)      # [128, 4]
    return dict(
        Wg=np.asarray(Wg).astype(np.float32),
        gWs=gWs, gB=gB,
        Wl=np.asarray(Wl).astype(np.float32),
        bl=np.tile(np.asarray(bl).astype(np.float32).reshape(1, N_CLS), (P, 1)),
    )


def _build_program(n_nodes, CA, CB, nw, npc_pad, split):
    import concourse.bacc as bacc
    import concourse.mybir as mybir
    from concourse.tile import TileContext

    f32 = mybir.dt.float32
    i16 = mybir.dt.int16
    AF = mybir.ActivationFunctionType
    ALU = mybir.AluOpType
    AX = mybir.AxisListType

    C = CA + CB
    K = nw * C
    nc = bacc.Bacc("TRN2")

    xin = nc.dram_tensor("xin", [n_nodes, F_IN], f32, kind="ExternalInput")
    idx_d = nc.dram_tensor("idx", [P, K * 8], i16, kind="ExternalInput")
    sval_d = nc.dram_tensor("sval", [P, K * P], f32, kind="ExternalInput")
    hT_d = nc.dram_tensor("hT", [P, npc_pad], f32, kind="ExternalInput")
    Wg_d = nc.dram_tensor("Wg", [P, F_IN], f32, kind="ExternalInput")
    gWs_d = nc.dram_tensor("gWs", [P, 6 * F_HID], f32, kind="ExternalInput")
    gB_d = nc.dram_tensor("gB", [P, 4], f32, kind="ExternalInput")
    Wl_d = nc.dram_tensor("Wl", [P, N_CLS], f32, kind="ExternalInput")
    bl_d = nc.dram_tensor("bl", [P, N_CLS], f32, kind="ExternalInput")

    probs_o = nc.dram_tensor("probs", [npc_pad, N_CLS], f32, kind="ExternalOutput")
    hnT_o = nc.dram_tensor("hnT", [P, npc_pad], f32, kind="ExternalOutput")

    with TileContext(nc) as tc:
        with (
            tc.tile_pool(name="const", bufs=1) as cp,
            tc.tile_pool(name="gpool", bufs=3) as gp,
            tc.tile_pool(name="spool", bufs=3) as sp,
            tc.tile_pool(name="work", bufs=2) as wp,
            tc.tile_pool(name="psA", bufs=2, space="PSUM") as psA,
            tc.tile_pool(name="psG", bufs=1, space="PSUM") as psG,
            tc.tile_pool(name="psL", bufs=2, space="PSUM") as psL,
        ):
            # ---- resident constants ----
            idx_sb = cp.tile([P, K * 8], i16, tag="idx")
            nc.sync.dma_start(idx_sb[:, :], idx_d[:, :])
            hT_sb = cp.tile([P, npc_pad], f32, tag="hT")
            nc.sync.dma_start(hT_sb[:, :], hT_d[:, :])
            Wg_sb = cp.tile([P, F_IN], f32, tag="Wg")
            nc.sync.dma_start(Wg_sb[:, :], Wg_d[:, :])
            gW = cp.tile([P, 6 * F_HID], f32, tag="gW")
            nc.sync.dma_start(gW[:, :], gWs_d[:, :])
            gB_sb = cp.tile([P, 4], f32, tag="gB")
            nc.sync.dma_start(gB_sb[:, :], gB_d[:, :])
            Wl_sb = cp.tile([P, N_CLS], f32, tag="Wl")
            nc.sync.dma_start(Wl_sb[:, :], Wl_d[:, :])
            bl_sb = cp.tile([P, N_CLS], f32, tag="bl")
            nc.sync.dma_start(bl_sb[:, :], bl_d[:, :])

            for w in range(nw):
                # ---- gather x rows for this window's entries (A/B halves) ----
                g = gp.tile([P, C, F_IN], f32, tag="g")
                nc.gpsimd.dma_gather(
                    g[:, 0:CA, :], xin[0:split, :],
                    idx_sb[:, w * C * 8:w * C * 8 + CA * 8],
                    CA * P, CA * P, F_IN, single_packet=False,
                )
                if CB:
                    nc.gpsimd.dma_gather(
                        g[:, CA:C, :], xin[split:n_nodes, :],
                        idx_sb[:, w * C * 8 + CA * 8:(w + 1) * C * 8],
                        CB * P, CB * P, F_IN, single_packet=False,
                    )
                s = sp.tile([P, C * P], f32, tag="s")
                nc.sync.dma_start(s[:, :], sval_d[:, w * C * P:(w + 1) * C * P])

                # ---- aggT[feat, node] = sum_k G_k.T @ S_k ----
                pa = psA.tile([P, P], f32, tag="agg")
                for k in range(C):
                    nc.tensor.matmul(
                        pa[:, :], g[:, k, :], s[:, k * P:(k + 1) * P],
                        start=(k == 0), stop=(k == C - 1),
                    )
                aggT = wp.tile([P, P], f32, tag="aggT")
                nc.vector.tensor_copy(aggT[:, :], pa[:, :])

                # ---- h.T = Wg.T @ aggT + bg ----
                ph = psG.tile([P, P], f32, tag="ph")
                nc.tensor.matmul(ph[:, :], Wg_sb[:, :], aggT[:, :], start=True, stop=True)
                hTt = wp.tile([P, P], f32, tag="h")
                nc.vector.tensor_scalar_add(hTt[:, :], ph[:, :], gB_sb[:, 0:1])

                Hs = hT_sb[:, w * P:(w + 1) * P]

                # ---- Z ----
                pz = psG.tile([P, P], f32, tag="pz")
                nc.tensor.matmul(pz[:, :], gW[:, 0:128], hTt[:, :], start=True, stop=False)
                nc.tensor.matmul(pz[:, :], gW[:, 128:256], Hs, start=False, stop=True)
                Z = wp.tile([P, P], f32, tag="Z")
                nc.scalar.activation(Z[:, :], pz[:, :], AF.Sigmoid, bias=gB_sb[:, 1:2])

                # ---- R ----
                pr = psG.tile([P, P], f32, tag="pr")
                nc.tensor.matmul(pr[:, :], gW[:, 256:384], hTt[:, :], start=True, stop=False)
                nc.tensor.matmul(pr[:, :], gW[:, 384:512], Hs, start=False, stop=True)
                R = wp.tile([P, P], f32, tag="R")
                nc.scalar.activation(R[:, :], pr[:, :], AF.Sigmoid, bias=gB_sb[:, 2:3])

                # ---- Ht ----
                HsR = wp.tile([P, P], f32, tag="HsR")
                nc.vector.tensor_tensor(HsR[:, :], Hs, R[:, :], op=ALU.mult)
                pt = psG.tile([P, P], f32, tag="pt")
                nc.tensor.matmul(pt[:, :], gW[:, 512:640], hTt[:, :], start=True, stop=False)
                nc.tensor.matmul(pt[:, :], gW[:, 640:768], HsR[:, :], start=False, stop=True)
                Ht = wp.tile([P, P], f32, tag="Ht")
                nc.scalar.activation(Ht[:, :], pt[:, :], AF.Tanh, bias=gB_sb[:, 3:4])

                # ---- Hn = Ht + Z*(Hs - Ht) ----
                d1 = wp.tile([P, P], f32, tag="d1")
                nc.vector.tensor_tensor(d1[:, :], Hs, Ht[:, :], op=ALU.subtract)
                d2 = wp.tile([P, P], f32, tag="d2")
                nc.vector.tensor_tensor(d2[:, :], Z[:, :], d1[:, :], op=ALU.mult)
                HnT = wp.tile([P, P], f32, tag="HnT")
                nc.vector.tensor_tensor(HnT[:, :], Ht[:, :], d2[:, :], op=ALU.add)
                nc.sync.dma_start(hnT_o[:, w * P:(w + 1) * P], HnT[:, :])

                # ---- logits (node-major) + softmax ----
                Ru = wp.tile([P, P], f32, tag="Ru")
                nc.scalar.activation(Ru[:, :], HnT[:, :], AF.Relu)
                pl = psL.tile([P, N_CLS], f32, tag="pl")
                nc.tensor.matmul(pl[:, :], Ru[:, :], Wl_sb[:, :], start=True, stop=True)
                lg = wp.tile([P, N_CLS], f32, tag="lg")
                nc.vector.tensor_tensor(lg[:, :], pl[:, :], bl_sb[:, :], op=ALU.add)
                negm = wp.tile([P, 1], f32, tag="negm")
                nc.vector.reduce_max(negm[:, :], lg[:, :], axis=AX.X, negate=True)
                e = wp.tile([P, N_CLS], f32, tag="e")
                nc.scalar.activation(e[:, :], lg[:, :], AF.Exp, bias=negm[:, 0:1])
                ssum = wp.tile([P, 1], f32, tag="ssum")
                nc.vector.reduce_sum(ssum[:, :], e[:, :], axis=AX.X)
                rinv = wp.tile([P, 1], f32, tag="rinv")
                nc.vector.reciprocal(rinv[:, :], ssum[:, :])
                pt_out = wp.tile([P, N_CLS], f32, tag="probs")
                nc.vector.tensor_scalar_mul(pt_out[:, :], e[:, :], rinv[:, 0:1])
                nc.sync.dma_start(probs_o[w * P:(w + 1) * P, :], pt_out[:, :])

    if not nc.is_finalized():
        nc.finalize()
    return nc


def kernel(x, edge_index, edge_weight, hidden1,
           Wg, bg, Wz0, Wz1, bz, Wr0, Wr1, br, Wh0, Wh1, bh, Wl, bl):
    from concourse.bass_utils import run_bass_kernel_spmd

    n_nodes = x.shape[0]
    pre = _preprocess(x, edge_index, edge_weight, hidden1, n_nodes, N_CORES)
    wts = _pack_weights(Wg, bg, Wz0, Wz1, bz, Wr0, Wr1, br, Wh0, Wh1, bh, Wl, bl)

    nc = _build_program(n_nodes, pre["CA"], pre["CB"], pre["nw"],
                        pre["npc_pad"], pre["split"])

    x_f = np.ascontiguousarray(np.asarray(x).astype(np.float32))
    in_maps = []
    for c in range(N_CORES):
        m = dict(xin=x_f, idx=pre["idx"][c], sval=pre["sval"][c], hT=pre["hT"][c])
        m.update(wts)
        in_maps.append(m)

    res = run_bass_kernel_spmd(nc, in_maps, core_ids=list(range(N_CORES)))
    npc = pre["npc"]
    probs = np.concatenate([np.asarray(r["probs"])[:npc] for r in res.results], axis=0)
    hn = np.concatenate([np.asarray(r["hnT"]).T[:npc] for r in res.results], axis=0)
    return probs, hn
